# revision 1
# baseline (speedup 1.0000x reference)
"""ChildSum TreeLSTM + attention, 8-core SPMD Trainium2 kernel.

Design (all vectors in "column form" [128, n_chunks] on SBUF):
  - mem_dim = 1024 split into 8 chunks of 128; core j owns state slice j.
  - Per step t (serial chain, two remote_dma sync points):
      phase A : gates pre-act for own slice = Wrec^T @ h  (32 stationary MMs)
      gates   : ACT sigmoid/tanh with xproj bias folded in; c/h update (DVE)
      w-gemv  : w_partial = W1[own rows]^T @ h_new_own    (8 MMs)
      SYNC1   : bcast [h_new placed via one-hot mask | w_partial]  [128,16]
      attn    : tT = tanh(hw2T + w bias) (8 ACT) ; scores (8 MMs, K=128)
                e' = exp(scores) [64,1]
      numer   : numer_partialT = H_own^T @ e'             (8 MMs, K=64)
      SYNC2   : bcast [numer_partialT | e' padded]        [128,9]
      h_att   : colsumH + h_new_full - numer_sum/Z  (DVE + PE ones-reduce)
  - Receiver-side sums are order-free => XOR slot permutation needs no core id.
  - x projections, HW2 = H @ W_attnh[1024:], colsumH precomputed on device.

Host prep does layout only (transpose/slice/cast) - zero FLOPs.
"""
import numpy as np
from contextlib import ExitStack

import concourse.bass as bass
import concourse.tile as tile
from concourse import bacc, mybir
from concourse.bass import create_sync_update
from concourse.tile_rust import add_dep_helper

F32 = mybir.dt.float32
AF = mybir.ActivationFunctionType
N_CORES = 8
MEM = 1024
IN_DIM = 1024
MROWS = 512          # attention rows
KC = MEM // 128      # 8 column chunks
RPC = MROWS // N_CORES  # 64 attention rows per core

# Sems incremented by remote cores / DMA completion that the Tile scheduling
# pass (single-core sim, no_exec) can never see. Pre-satisfied there only;
# the runtime NEFF keeps the real waits.
_EXTERNAL_SEMS: list = []
_OrigCoreSim = tile.CoreSim


class _SchedCoreSim(_OrigCoreSim):
    def __init__(self, *a, **kw):
        super().__init__(*a, **kw)
        for sem in _EXTERNAL_SEMS:
            self.update_semaphore(create_sync_update(sem, 1 << 22))


tile.CoreSim = _SchedCoreSim


def _bcast_all(nc, out_slot_of, in_ap, remote_sem, local_sem, chain):
    """8 single-dest relative broadcasts: slot k -> peer (own_tpb ^ k).

    The SWDGE descriptor ring is a FIFO fired in ring order by trigger_dma,
    so the preps of consecutive syncs must execute on Q7 in trace order.
    `chain` holds the previous prep/trigger instruction; every prep and the
    trigger get a no_sync dep on it.
    """
    prev = chain[0]
    for k in range(N_CORES):
        rdests = [None] * N_CORES
        rdests[k] = (0, k)
        inst = nc.gpsimd.remote_dma_broadcast(
            out_ap=out_slot_of(k),
            in_ap=in_ap,
            remote_sem=remote_sem,
            local_sem=local_sem,
            rdests=rdests,
        )
        if prev is not None:
            add_dep_helper(inst.ins, prev, False, "swdge ring order")
        prev = inst.ins
    trig = nc.gpsimd.trigger_dma(count=None)
    add_dep_helper(trig.ins, prev, False, "swdge ring order")
    chain[0] = trig.ins


def build_nc(T: int, wdt=mybir.dt.float32, t_run: int | None = None, no_comm: bool = False):
    """Build the SPMD program. wdt: dtype for the big stationary weights."""
    del _EXTERNAL_SEMS[:]
    nc = bacc.Bacc()

    # ---- DRAM parameters (per-core tensors supplied via in_maps) ----
    dp = lambda n, s, dt=F32: nc.declare_dram_parameter(n, s, dt, isOutput=False)
    xT = dp("xT", [128, KC * T])            # xT[p, T*k+t] = X[t, 128k+p]
    wx = dp("wx", [128, 4 * KC * 128])      # tile(g,k): Wx_g[128k+a, 128j+b]
    wrec = dp("wrec", [128, 4 * KC * 128], wdt)
    w1t = dp("w1t", [128, KC * 128], wdt)   # tile m: W1[128j+a, 128m+b]
    w2t = dp("w2t", [128, KC * KC * 128])   # tile(c,k): W2[128k+a, 128c+b]
    hTs = dp("hT", [128, KC * MROWS])       # hT[p, 512m+i] = H[i, 128m+p]
    hTown = dp("hTown", [128, KC * RPC])    # hTown[p, 64k+i] = H[64j+i, 128k+p]
    hrows = dp("hrows", [RPC, MEM], wdt)    # H[64j+a, b]
    wa = dp("wa", [128, KC])                # wa[p,c] = Wa[128c+p]
    bias_x = dp("bias_x", [128, 4])         # per-gate bias for own slice
    bias2 = dp("bias2", [128, KC])          # b_attnh column form
    mask = dp("mask", [128, KC])            # one-hot col own_core
    hout = nc.declare_dram_parameter("hout", [T, 128, KC], F32, isOutput=True)

    with tile.TileContext(nc) as tc, ExitStack() as ctx:
        sem1 = ctx.enter_context(nc.semaphore("rdma_sem1"))
        sem2 = ctx.enter_context(nc.semaphore("rdma_sem2"))
        lsem1 = ctx.enter_context(nc.semaphore("rdma_lsem1"))
        lsem2 = ctx.enter_context(nc.semaphore("rdma_lsem2"))
        _EXTERNAL_SEMS.extend([sem1, sem2, lsem1, lsem2])

        # ---------- persistent buffers ----------
        comm = ctx.enter_context(tc.tile_pool(name="comm", bufs=1))
        pay1 = [comm.tile([128, 16], F32, name=f"pay1_{p}", tag=f"pay1_{p}") for p in range(2)]
        rec1 = [comm.tile([128, 128], F32, name=f"rec1_{p}", tag=f"rec1_{p}") for p in range(2)]
        pay2 = [comm.tile([128, 9], F32, name=f"pay2_{p}", tag=f"pay2_{p}") for p in range(2)]
        rec2 = [comm.tile([128, 72], F32, name=f"rec2_{p}", tag=f"rec2_{p}") for p in range(2)]

        const = ctx.enter_context(tc.tile_pool(name="const", bufs=1))
        wrec_sb = const.tile([128, 4 * KC * 128], wdt, tag="wrec")
        w1t_sb = const.tile([128, KC * 128], wdt, tag="w1t")
        hrows_sb = const.tile([RPC, MEM], wdt, tag="hrows")
        wa_sb = const.tile([128, KC], F32, tag="wa")
        hw2T_sb = const.tile([128, KC * RPC], F32, tag="hw2T")
        xproj_sb = const.tile([128, 4 * T], F32, tag="xproj")
        csum_sb = const.tile([128, KC], F32, tag="csum")
        ones_sb = const.tile([128, 128], F32, tag="ones")
        mask_sb = const.tile([128, KC], F32, tag="mask")

        nc.sync.dma_start(wrec_sb[:, :], wrec.ap())
        nc.sync.dma_start(w1t_sb[:, :], w1t.ap())
        nc.sync.dma_start(hrows_sb[:, :], hrows.ap())
        nc.sync.dma_start(wa_sb[:, :], wa.ap())
        nc.sync.dma_start(mask_sb[:, :], mask.ap())
        nc.vector.memset(ones_sb[:, :], 1.0)
        for p in range(2):
            nc.vector.memset(pay2[p][:, :], 0.0)
        if no_comm:
            for p in range(2):
                nc.vector.memset(rec1[p][:, :], 1.0)
                nc.vector.memset(rec2[p][:, :], 1.0)

        # ---------- device precompute ----------
        with tc.tile_pool(name="pre", bufs=1) as pre, \
             tc.tile_pool(name="prepsum", bufs=1, space="PSUM") as pps:
            xT_sb = pre.tile([128, KC * T], F32, tag="xT")
            wx_sb = pre.tile([128, 4 * KC * 128], F32, tag="wx")
            w2t_sb = pre.tile([128, KC * KC * 128], F32, tag="w2t")
            hT_sb = pre.tile([128, KC * MROWS], F32, tag="hT")
            hTown_sb = pre.tile([128, KC * RPC], F32, tag="hTown")
            bx_sb = pre.tile([128, 4], F32, tag="bias_x")
            b2_sb = pre.tile([128, KC], F32, tag="bias2")
            nc.sync.dma_start(xT_sb[:, :], xT.ap())
            nc.sync.dma_start(wx_sb[:, :], wx.ap())
            nc.sync.dma_start(w2t_sb[:, :], w2t.ap())
            nc.sync.dma_start(hT_sb[:, :], hTs.ap())
            nc.sync.dma_start(hTown_sb[:, :], hTown.ap())
            nc.sync.dma_start(bx_sb[:, :], bias_x.ap())
            nc.sync.dma_start(b2_sb[:, :], bias2.ap())

            # xproj[g]: [128, T] = sum_k Wx_g[k-chunk]^T @ xT[k-chunk]
            for g in range(4):
                ps = pps.tile([128, T], F32, tag="ps_x")
                for k in range(KC):
                    nc.tensor.matmul(
                        ps[:, :],
                        wx_sb[:, (g * KC + k) * 128:(g * KC + k + 1) * 128],
                        xT_sb[:, k * T:(k + 1) * T],
                        start=(k == 0), stop=(k == KC - 1),
                    )
                # xproj col layout: [:, T*g + t]; add per-gate bias
                nc.vector.tensor_scalar_add(
                    xproj_sb[:, g * T:(g + 1) * T], ps[:, :], bx_sb[:, g:g + 1]
                )

            # hw2T c-chunk: [128, 64] = sum_k W2[c,k]^T @ hT[k, own rows]
            for c in range(KC):
                ps2 = pps.tile([128, RPC], F32, tag="ps_h")
                for k in range(KC):
                    nc.tensor.matmul(
                        ps2[:, :],
                        w2t_sb[:, (c * KC + k) * 128:(c * KC + k + 1) * 128],
                        hTown_sb[:, k * RPC:(k + 1) * RPC],
                        start=(k == 0), stop=(k == KC - 1),
                    )
                nc.vector.tensor_scalar_add(
                    hw2T_sb[:, c * RPC:(c + 1) * RPC], ps2[:, :], b2_sb[:, c:c + 1]
                )

            # colsumH column form: reduce hT chunks along free dim
            for m in range(KC):
                nc.vector.reduce_sum(
                    csum_sb[:, m:m + 1],
                    hT_sb[:, m * MROWS:(m + 1) * MROWS],
                    axis=mybir.AxisListType.X,
                )

        # ---------- state & per-step pools ----------
        sp = ctx.enter_context(tc.tile_pool(name="step", bufs=2))
        psp = ctx.enter_context(tc.tile_pool(name="spsum", bufs=1, space="PSUM"))

        chain = [None]
        hcol = sp.tile([128, KC], F32, tag="hcol")
        ccol = sp.tile([128, 1], F32, tag="ccol")
        nc.vector.memset(hcol[:, :], 0.0)
        nc.vector.memset(ccol[:, :], 0.0)

        for t in range(t_run if t_run is not None else T):
            par = t & 1
            # ---- phase A: gate pre-activations for own slice ----
            if wdt != F32:
                hcol_w = sp.tile([128, KC], wdt, tag="hcol_w")
                nc.vector.tensor_copy(hcol_w[:, :], hcol[:, :])
            else:
                hcol_w = hcol
            psA = psp.tile([128, 4], F32, tag="psA")
            for g in range(4):
                for k in range(KC):
                    nc.tensor.matmul(
                        psA[:, g:g + 1],
                        wrec_sb[:, (g * KC + k) * 128:(g * KC + k + 1) * 128],
                        hcol_w[:, k:k + 1],
                        start=(k == 0), stop=(k == KC - 1),
                    )
            # ---- gates (xproj folded in as ACT bias) ----
            gates = sp.tile([128, 4], F32, tag="gates")
            for g, fn in ((0, AF.Sigmoid), (1, AF.Sigmoid), (2, AF.Tanh), (3, AF.Sigmoid)):
                nc.scalar.activation(
                    gates[:, g:g + 1], psA[:, g:g + 1], fn,
                    bias=xproj_sb[:, g * T + t:g * T + t + 1],
                )
            iu = sp.tile([128, 1], F32, tag="iu")
            nc.vector.tensor_mul(iu[:, :], gates[:, 0:1], gates[:, 2:3])
            ccol_new = sp.tile([128, 1], F32, tag="ccol")
            nc.vector.tensor_mul(ccol_new[:, :], gates[:, 3:4], ccol[:, :])
            nc.vector.tensor_add(ccol_new[:, :], ccol_new[:, :], iu[:, :])
            ccol = ccol_new
            tanh_c = sp.tile([128, 1], F32, tag="tanh_c")
            nc.scalar.activation(tanh_c[:, :], ccol[:, :], AF.Tanh)
            h_new = sp.tile([128, 1], F32, tag="h_new")
            hn_inst = nc.vector.tensor_mul(h_new[:, :], gates[:, 1:2], tanh_c[:, :])

            # place h_new into payload col own_core via one-hot mask
            # (mask read from const pool each step; mask dram loaded once)
            # w-gemv: w_partialT = W1[own]^T @ h_new
            psW = psp.tile([128, KC], F32, tag="psW")
            if wdt != F32:
                h_new_w = sp.tile([128, 1], wdt, tag="h_new_w")
                nc.vector.tensor_copy(h_new_w[:, :], h_new[:, :])
            else:
                h_new_w = h_new
            for m in range(KC):
                nc.tensor.matmul(
                    psW[:, m:m + 1],
                    w1t_sb[:, m * 128:(m + 1) * 128],
                    h_new_w[:, :],
                    start=True, stop=True,
                )
            if t >= 2 and not no_comm:
                lw1 = nc.vector.wait_ge(lsem1, 128 * t)
                add_dep_helper(lw1.ins, hn_inst.ins, False, "anchor lsem1 wait")
            mm_inst = nc.vector.tensor_scalar_mul(pay1[par][:, 0:8], mask_sb[:, :], h_new[:, :])
            cp1 = nc.vector.tensor_copy(pay1[par][:, 8:16], psW[:, :])
            if t >= 2 and not no_comm:
                add_dep_helper(mm_inst.ins, lw1.ins, False, "pay1 WAR")
                add_dep_helper(cp1.ins, lw1.ins, False, "pay1 WAR")

            # ---- SYNC 1 ----
            if not no_comm:
                _bcast_all(nc, lambda k: rec1[par][:, k * 16:(k + 1) * 16],
                           pay1[par][:, 0:16], sem1, lsem1, chain)
                w1_inst = nc.vector.wait_ge(sem1, 16 * (t + 1))
                add_dep_helper(w1_inst.ins, cp1.ins, False, "anchor sem1 wait in step")

            hnew_full = sp.tile([128, KC], F32, tag="hnew_full")
            r1 = rec1[par][:, :].rearrange("p (s c) -> p c s", s=N_CORES)
            i1 = nc.vector.reduce_sum(hnew_full[:, :], r1[:, 0:8, :], axis=mybir.AxisListType.X)
            w_sum = sp.tile([128, KC], F32, tag="w_sum")
            i2 = nc.vector.reduce_sum(w_sum[:, :], r1[:, 8:16, :], axis=mybir.AxisListType.X)
            if not no_comm:
                add_dep_helper(i1.ins, w1_inst.ins, False, "gate recv1 on sem1")
                add_dep_helper(i2.ins, w1_inst.ins, False, "gate recv1 on sem1")

            # ---- attention rows (own 64): tT = tanh(hw2T + w bias) ----
            tT = sp.tile([128, KC * RPC], F32, tag="tT")
            tanh_insts = []
            for c in range(KC):
                tanh_insts.append(nc.scalar.activation(
                    tT[:, c * RPC:(c + 1) * RPC],
                    hw2T_sb[:, c * RPC:(c + 1) * RPC],
                    AF.Tanh, bias=w_sum[:, c:c + 1],
                ))
            psS = psp.tile([64, 1], F32, tag="psS")
            for c in range(KC):
                nc.tensor.matmul(
                    psS[:, :], tT[:, c * RPC:(c + 1) * RPC], wa_sb[:, c:c + 1],
                    start=(c == 0), stop=(c == KC - 1),
                )
            if t >= 2 and not no_comm:
                lw2a = nc.scalar.wait_ge(lsem2, 128 * t)
                add_dep_helper(lw2a.ins, tanh_insts[-1].ins, False, "anchor lsem2 act wait")
            e_inst = nc.scalar.activation(pay2[par][0:64, 8:9], psS[:, :], AF.Exp)
            if t >= 2 and not no_comm:
                add_dep_helper(e_inst.ins, lw2a.ins, False, "pay2 WAR act")

            # ---- numer partial: H_own^T @ e'  (K = 64) ----
            psN = psp.tile([128, KC], F32, tag="psN")
            if wdt != F32:
                e_w = sp.tile([64, 1], wdt, tag="e_w")
                nc.vector.tensor_copy(e_w[:, :], pay2[par][0:64, 8:9])
                e_rhs = e_w[:, :]
            else:
                e_rhs = pay2[par][0:64, 8:9]
            for m in range(KC):
                nc.tensor.matmul(
                    psN[:, m:m + 1], hrows_sb[:, m * 128:(m + 1) * 128], e_rhs,
                    start=True, stop=True,
                )
            if t >= 2 and not no_comm:
                lw2v = nc.vector.wait_ge(lsem2, 128 * t)
                add_dep_helper(lw2v.ins, i2.ins, False, "anchor lsem2 dve wait")
            cp2 = nc.vector.tensor_copy(pay2[par][:, 0:8], psN[:, :])
            if t >= 2 and not no_comm:
                add_dep_helper(cp2.ins, lw2v.ins, False, "pay2 WAR dve")

            # ---- SYNC 2 ----
            if not no_comm:
                _bcast_all(nc, lambda k: rec2[par][:, k * 9:(k + 1) * 9],
                           pay2[par][:, 0:9], sem2, lsem2, chain)
                w2_inst = nc.vector.wait_ge(sem2, 16 * (t + 1))
                add_dep_helper(w2_inst.ins, cp2.ins, False, "anchor sem2 wait in step")

            numer = sp.tile([128, KC], F32, tag="numer")
            r2 = rec2[par][:, :].rearrange("p (s c) -> p c s", s=N_CORES)
            i3 = nc.vector.reduce_sum(numer[:, :], r2[:, 0:8, :], axis=mybir.AxisListType.X)
            zcol = sp.tile([128, 1], F32, tag="zcol")
            i4 = nc.vector.reduce_sum(zcol[:, :], r2[:, 8:9, :], axis=mybir.AxisListType.X)
            if not no_comm:
                add_dep_helper(i3.ins, w2_inst.ins, False, "gate recv2 on sem2")
                add_dep_helper(i4.ins, w2_inst.ins, False, "gate recv2 on sem2")

            # Z broadcast over partitions via ones matmul, then 1/Z
            psZ = psp.tile([128, 1], F32, tag="psZ")
            nc.tensor.matmul(psZ[:, :], ones_sb[:, :], zcol[:, :], start=True, stop=True)
            rz = sp.tile([128, 1], F32, tag="rz")
            nc.vector.reciprocal(rz[:, :], psZ[:, :])

            # h_att = csum + hnew_full - numer * rz
            sub = sp.tile([128, KC], F32, tag="sub")
            nc.vector.tensor_scalar_mul(sub[:, :], numer[:, :], rz[:, :])
            hcol_new = sp.tile([128, KC], F32, tag="hcol")
            nc.vector.tensor_add(hcol_new[:, :], hnew_full[:, :], csum_sb[:, :])
            nc.vector.tensor_sub(hcol_new[:, :], hcol_new[:, :], sub[:, :])
            hcol = hcol_new

            nc.sync.dma_start(hout.ap()[t], hcol[:, :])

    nc.compile()
    return nc


def prep_in_maps(inputs: dict, T: int, wdt_np=np.float32) -> list[dict]:
    """Host-side layout-only prep of per-core input maps."""
    X = np.asarray(inputs["inputs"], np.float32).reshape(T, IN_DIM)
    H = np.asarray(inputs["hiddn_state_mat"], np.float32)
    W_ioux = np.asarray(inputs["W_ioux"], np.float32)
    W_iouh = np.asarray(inputs["W_iouh"], np.float32)
    W_fx = np.asarray(inputs["W_fx"], np.float32)
    W_fh = np.asarray(inputs["W_fh"], np.float32)
    Wa = np.asarray(inputs["Wa"], np.float32).reshape(MEM)
    W_attnh = np.asarray(inputs["W_attnh"], np.float32)
    b_iou = (np.asarray(inputs["b_ioux"], np.float32)
             + np.asarray(inputs["b_iouh"], np.float32))
    b_f = (np.asarray(inputs["b_fx"], np.float32)
           + np.asarray(inputs["b_fh"], np.float32))
    b_attnh = np.asarray(inputs["b_attnh"], np.float32)

    W1 = W_attnh[:MEM]
    W2 = W_attnh[MEM:]

    # replicated tensors
    xT_l = X.T.reshape(KC, 128, T).transpose(1, 0, 2).reshape(128, KC * T)
    xT_l = np.ascontiguousarray(xT_l)
    # w2t tile(c,k) = W2[128k+a, 128c+b] -> [128, 64*128]
    w2t = np.zeros((128, KC * KC * 128), np.float32)
    for c in range(KC):
        for k in range(KC):
            w2t[:, (c * KC + k) * 128:(c * KC + k + 1) * 128] = \
                W2[128 * k:128 * (k + 1), 128 * c:128 * (c + 1)]
    hT_l = np.ascontiguousarray(
        H.T.reshape(KC, 128, MROWS).transpose(1, 0, 2).reshape(128, KC * MROWS))
    wa_l = np.ascontiguousarray(Wa.reshape(KC, 128).T)
    b2_l = np.ascontiguousarray(b_attnh.reshape(KC, 128).T)

    gate_w = [W_iouh[:, 0:MEM], W_iouh[:, MEM:2 * MEM], W_iouh[:, 2 * MEM:], W_fh]
    gate_wx = [W_ioux[:, 0:MEM], W_ioux[:, MEM:2 * MEM], W_ioux[:, 2 * MEM:], W_fx]
    gate_b = [b_iou[0:MEM], b_iou[MEM:2 * MEM], b_iou[2 * MEM:], b_f]

    maps = []
    for j in range(N_CORES):
        wrec = np.zeros((128, 4 * KC * 128), np.float32)
        wx = np.zeros((128, 4 * KC * 128), np.float32)
        for g in range(4):
            for k in range(KC):
                sl = np.s_[:, (g * KC + k) * 128:(g * KC + k + 1) * 128]
                wrec[sl] = gate_w[g][128 * k:128 * (k + 1), 128 * j:128 * (j + 1)]
                wx[sl] = gate_wx[g][128 * k:128 * (k + 1), 128 * j:128 * (j + 1)]
        w1t = np.zeros((128, KC * 128), np.float32)
        for m in range(KC):
            w1t[:, m * 128:(m + 1) * 128] = \
                W1[128 * j:128 * (j + 1), 128 * m:128 * (m + 1)]
        bias_x = np.stack([gate_b[g][128 * j:128 * (j + 1)] for g in range(4)], axis=1)
        mask = np.zeros((128, KC), np.float32)
        mask[:, j] = 1.0
        Hown = H[RPC * j:RPC * (j + 1)]            # [64, 1024]
        hTown_l = np.ascontiguousarray(
            Hown.T.reshape(KC, 128, RPC).transpose(1, 0, 2).reshape(128, KC * RPC))
        maps.append({
            "xT": xT_l, "wx": wx,
            "wrec": wrec.astype(wdt_np), "w1t": w1t.astype(wdt_np),
            "w2t": w2t, "hT": hT_l, "hTown": hTown_l,
            "hrows": np.ascontiguousarray(H[RPC * j:RPC * (j + 1)]).astype(wdt_np),
            "wa": wa_l, "bias_x": np.ascontiguousarray(bias_x),
            "bias2": b2_l, "mask": mask,
        })
    return maps


def postprocess(hout_core0: np.ndarray, T: int) -> np.ndarray:
    # hout [T, 128, KC]: [t, p, c] = h_att_t[128c + p]
    return np.ascontiguousarray(
        hout_core0.transpose(0, 2, 1).reshape(T, MEM)).astype(np.float32)


# ----------------------------------------------------------------------------
# Harness entry point: full (unsharded) inputs -> full output.
# ----------------------------------------------------------------------------
def kernel(**inputs) -> np.ndarray:
    from concourse.bass_utils import run_bass_kernel_spmd

    T = int(np.asarray(inputs["inputs"]).shape[0])
    nc = build_nc(T)
    in_maps = prep_in_maps(inputs, T)
    res = run_bass_kernel_spmd(nc, in_maps, core_ids=list(range(N_CORES)))
    hout = np.asarray(res.results[0]["hout"]).reshape(T, 128, KC)
    return postprocess(hout, T)



# revision 2
# speedup vs baseline: 1.1387x; 1.1387x over previous
"""ChildSum TreeLSTM + attention, 8-core SPMD Trainium2 kernel — ONE sync/step.

Core j owns mem-dim slice j of the LSTM state and outputs h_att for it.
Per step t:
  phase A : psA = [xproj + b + Wrec^T csum] + Wrec^T h_new(t-1) + Gn ê(t-1)
            (Wrec^T h_att(t-1) decomposed via h_att = csum + h_new - H^T ê;
             Gn = -(H @ Wrec_j)^T precomputed on device; ê from last step's
             replicated attention, so NO second sync is needed)
  gates   : ACT sigmoid/tanh (xproj bias); c/h update (DVE) -> h_new slice
  w-gemv  : w_partial = W1[own K-slice]^T h_new_own (8 MMs)
  SYNC    : bcast [h_new one-hot | w_partial] [128,16]  (the ONLY sync/step)
  attn    : replicated on every core over ALL 512 rows:
            tT = tanh(hw2T_full + w_sum bias); scores [128,4] (32 MMs);
            e = exp via tanh identity; Z by ones-matmul; ê = e/Z
  h_att   : own slice = csum_own + h_new - (H[:, own]^T ê)  (4 MMs)
  output  : per-core hout [T,128,1]; host assembles the 8 slices.

The communication is exactly the baseline's proven pattern (8 single-dest
relative broadcasts + trigger + one arrival wait, 2 semaphores) -- just once
per step instead of twice. exp stays on the sigmoid/tanh ACT table set;
recurrent weights and the score path run in bfloat16 (tolerance 2e-2).
"""
import numpy as np
from contextlib import ExitStack

import concourse.bass as bass
import concourse.tile as tile
from concourse import bacc, mybir
from concourse.bass import create_sync_update
from concourse.tile_rust import add_dep_helper

F32 = mybir.dt.float32
BF16 = mybir.dt.bfloat16
AF = mybir.ActivationFunctionType
N_CORES = 8
MEM = 1024
IN_DIM = 1024
MROWS = 512
KC = MEM // 128       # 8 dim chunks
RC = MROWS // 128     # 4 row chunks

_EXTERNAL_SEMS: list = []
_OrigCoreSim = tile.CoreSim


class _SchedCoreSim(_OrigCoreSim):
    def __init__(self, *a, **kw):
        super().__init__(*a, **kw)
        for sem in _EXTERNAL_SEMS:
            self.update_semaphore(create_sync_update(sem, 1 << 22))


tile.CoreSim = _SchedCoreSim


def _bcast_all(nc, out_slot_of, in_ap, remote_sem, local_sem, chain):
    """8 single-dest relative broadcasts (slot k -> peer own^k) + trigger."""
    prev = chain[0]
    for k in range(N_CORES):
        rdests = [None] * N_CORES
        rdests[k] = (0, k)
        inst = nc.gpsimd.remote_dma_broadcast(
            out_ap=out_slot_of(k),
            in_ap=in_ap,
            remote_sem=remote_sem,
            local_sem=local_sem,
            rdests=rdests,
        )
        if prev is not None:
            add_dep_helper(inst.ins, prev, False, "swdge ring order")
        prev = inst.ins
    trig = nc.gpsimd.trigger_dma(count=None)
    add_dep_helper(trig.ins, prev, False, "swdge ring order")
    chain[0] = trig.ins


def build_nc(T: int, wdt=BF16, t_run: int | None = None, no_comm: bool = False):
    del _EXTERNAL_SEMS[:]
    nc = bacc.Bacc()

    dp = lambda n, s, dt=F32: nc.declare_dram_parameter(n, s, dt, isOutput=False)
    xT = dp("xT", [128, KC * T])             # xT[p, T*k+t] = X[t, 128k+p]
    wx = dp("wx", [128, 4 * KC * 128])       # tile(g,k): Wx_g[128k+a, own cols]
    wrec = dp("wrec", [128, 4 * KC * 128], wdt)
    w1t = dp("w1t", [128, KC * 128], wdt)    # tile m: W1[own 128, 128m..]
    w2t = dp("w2t", [128, KC * KC * 128])    # tile(c,k): W2[128k+a, 128c+b]
    hTs = dp("hT", [128, KC * MROWS])        # hT[p, 512m+i] = H[i, 128m+p]
    h4own = dp("h4own", [128, RC * 128], wdt)  # [p, rc*128+q] = H[128rc+p, 128j+q]
    wa = dp("wa", [128, KC], wdt)            # 0.5 * Wa column form
    bias_x = dp("bias_x", [128, 4])
    bias2 = dp("bias2", [128, KC])
    mask = dp("mask", [128, KC])             # one-hot col own_core
    hout = nc.declare_dram_parameter("hout", [T, 128, 1], F32, isOutput=True)

    with tile.TileContext(nc) as tc, ExitStack() as ctx:
        sem1 = ctx.enter_context(nc.semaphore("rdma_sem1"))
        lsem1 = ctx.enter_context(nc.semaphore("rdma_lsem1"))
        _EXTERNAL_SEMS.extend([sem1, lsem1])

        cpool = ctx.enter_context(tc.tile_pool(name="comm", bufs=1))
        pay1 = [cpool.tile([128, 16], F32, name=f"pay1_{p}", tag=f"pay1_{p}") for p in range(2)]
        rec1 = [cpool.tile([128, 128], F32, name=f"rec1_{p}", tag=f"rec1_{p}") for p in range(2)]

        const = ctx.enter_context(tc.tile_pool(name="const", bufs=1))
        wrec_sb = const.tile([128, 4 * KC * 128], wdt, tag="wrec")
        w1t_sb = const.tile([128, KC * 128], wdt, tag="w1t")
        h4own_sb = const.tile([128, RC * 128], wdt, tag="h4own")
        wa_sb = const.tile([128, KC], wdt, tag="wa")
        # hw2T_full[p, c*512 + r] = (H @ W2)[r, 128c+p] + b2[128c+p]
        hw2T_sb = const.tile([128, KC * MROWS], F32, tag="hw2T")
        # Gn tile (g, rc): [128 rows-part, 128 outs] = -(H_rc @ Wrec_g), bf16
        gn_sb = const.tile([128, 4 * RC * 128], wdt, tag="gn")
        xproj_sb = const.tile([128, 4 * T], F32, tag="xproj")
        csum_sb = const.tile([128, KC], F32, tag="csum")
        csumw_sb = const.tile([128, KC], wdt, tag="csumw")
        csown_sb = const.tile([128, 1], F32, tag="csown")
        ones_sb = const.tile([128, 128], F32, tag="ones")
        mask_sb = const.tile([128, KC], F32, tag="mask")
        neg1_sb = const.tile([128, 1], F32, tag="neg1")

        nc.sync.dma_start(wrec_sb[:, :], wrec.ap())
        nc.sync.dma_start(w1t_sb[:, :], w1t.ap())
        nc.sync.dma_start(h4own_sb[:, :], h4own.ap())
        nc.sync.dma_start(wa_sb[:, :], wa.ap())
        nc.sync.dma_start(mask_sb[:, :], mask.ap())
        nc.vector.memset(ones_sb[:, :], 1.0)
        nc.vector.memset(neg1_sb[:, :], -1.0)
        if no_comm:
            for p in range(2):
                nc.vector.memset(rec1[p][:, :], 1.0)

        # ---------- device precompute ----------
        with tc.tile_pool(name="pre", bufs=1) as pre, \
             tc.tile_pool(name="prepsum", bufs=1, space="PSUM") as pps:
            xT_sb = pre.tile([128, KC * T], F32, tag="xT")
            wx_sb = pre.tile([128, 4 * KC * 128], F32, tag="wx")
            w2t_sb = pre.tile([128, KC * KC * 128], F32, tag="w2t")
            hT_sb = pre.tile([128, KC * MROWS], F32, tag="hT")
            bx_sb = pre.tile([128, 4], F32, tag="bias_x")
            b2_sb = pre.tile([128, KC], F32, tag="bias2")
            nc.sync.dma_start(xT_sb[:, :], xT.ap())
            nc.sync.dma_start(wx_sb[:, :], wx.ap())
            nc.sync.dma_start(w2t_sb[:, :], w2t.ap())
            nc.sync.dma_start(hT_sb[:, :], hTs.ap())
            nc.sync.dma_start(bx_sb[:, :], bias_x.ap())
            nc.sync.dma_start(b2_sb[:, :], bias2.ap())

            # colsumH column form; own-slice column; bf16 copy
            for m in range(KC):
                nc.vector.reduce_sum(
                    csum_sb[:, m:m + 1],
                    hT_sb[:, m * MROWS:(m + 1) * MROWS],
                    axis=mybir.AxisListType.X,
                )
            nc.vector.tensor_copy(csumw_sb[:, :], csum_sb[:, :])
            cm = pre.tile([128, KC], F32, tag="cm")
            nc.vector.tensor_mul(cm[:, :], csum_sb[:, :], mask_sb[:, :])
            nc.vector.reduce_sum(csown_sb[:, :], cm[:, :], axis=mybir.AxisListType.X)

            # gatesc[:, g] = (Wrec^T csum)[own outs, g]
            psg = pps.tile([128, 4], F32, tag="ps_g")
            for g in range(4):
                for k in range(KC):
                    nc.tensor.matmul(
                        psg[:, g:g + 1],
                        wrec_sb[:, (g * KC + k) * 128:(g * KC + k + 1) * 128],
                        csumw_sb[:, k:k + 1],
                        start=(k == 0), stop=(k == KC - 1),
                    )

            # xproj[g] = sum_k Wx_g[k]^T xT[k] + b_x[g] + gatesc[g]
            for g in range(4):
                ps = pps.tile([128, T], F32, tag="ps_x")
                for k in range(KC):
                    nc.tensor.matmul(
                        ps[:, :],
                        wx_sb[:, (g * KC + k) * 128:(g * KC + k + 1) * 128],
                        xT_sb[:, k * T:(k + 1) * T],
                        start=(k == 0), stop=(k == KC - 1),
                    )
                nc.vector.tensor_scalar_add(
                    xproj_sb[:, g * T:(g + 1) * T], ps[:, :], bx_sb[:, g:g + 1]
                )
                nc.vector.tensor_scalar_add(
                    xproj_sb[:, g * T:(g + 1) * T],
                    xproj_sb[:, g * T:(g + 1) * T], psg[:, g:g + 1]
                )

            # hw2T_full c-chunk: [128, 512] = sum_k W2[c,k]^T @ hT[k]
            for c in range(KC):
                ps2 = pps.tile([128, MROWS], F32, tag="ps_h")
                for k in range(KC):
                    nc.tensor.matmul(
                        ps2[:, :],
                        w2t_sb[:, (c * KC + k) * 128:(c * KC + k + 1) * 128],
                        hT_sb[:, k * MROWS:(k + 1) * MROWS],
                        start=(k == 0), stop=(k == KC - 1),
                    )
                nc.vector.tensor_scalar_add(
                    hw2T_sb[:, c * MROWS:(c + 1) * MROWS], ps2[:, :], b2_sb[:, c:c + 1]
                )

            # Gn(g, rc) = -(H_rc @ Wrec_g)  [128 rows, 128 outs]
            hTw_sb = pre.tile([128, KC * MROWS], wdt, tag="hTw")
            nc.vector.tensor_copy(hTw_sb[:, :], hT_sb[:, :])
            for g in range(4):
                for rc in range(RC):
                    psG = pps.tile([128, 128], F32, tag="ps_G")
                    for m in range(KC):
                        nc.tensor.matmul(
                            psG[:, :],
                            hTw_sb[:, m * MROWS + rc * 128:m * MROWS + (rc + 1) * 128],
                            wrec_sb[:, (g * KC + m) * 128:(g * KC + m + 1) * 128],
                            start=(m == 0), stop=(m == KC - 1),
                        )
                    nc.vector.tensor_scalar_mul(
                        gn_sb[:, (g * RC + rc) * 128:(g * RC + rc + 1) * 128],
                        psG[:, :], neg1_sb[:, :],
                    )

        # ---------- per-step ----------
        sp = ctx.enter_context(tc.tile_pool(name="step", bufs=2))
        psp = ctx.enter_context(tc.tile_pool(name="spsum", bufs=1, space="PSUM"))

        chain = [None]
        ehat = sp.tile([128, RC], wdt, tag="ehat")
        nc.vector.memset(ehat[:, :], 0.0)
        ccol = sp.tile([128, 1], F32, tag="ccol")
        nc.vector.memset(ccol[:, :], 0.0)
        # t=0 has h=0, but xproj carries the +Wrec^T csum fold. Start hnw at
        # -csum so Wrec^T(hnw) cancels it exactly; ehat=0 kills the G term.
        hnw = sp.tile([128, KC], wdt, tag="hnw")
        nc.vector.tensor_scalar_mul(hnw[:, :], csumw_sb[:, :], neg1_sb[:, :])

        for t in range(t_run if t_run is not None else T):
            par = t & 1
            tm = t % T
            # ---- phase A: psA = Wrec^T hnw + Gn ehat  (+ xproj' bias) ----
            psA = psp.tile([128, 4], F32, tag="psA")
            for g in range(4):
                for k in range(KC):
                    nc.tensor.matmul(
                        psA[:, g:g + 1],
                        wrec_sb[:, (g * KC + k) * 128:(g * KC + k + 1) * 128],
                        hnw[:, k:k + 1],
                        start=(k == 0), stop=False,
                    )
                for rc in range(RC):
                    nc.tensor.matmul(
                        psA[:, g:g + 1],
                        gn_sb[:, (g * RC + rc) * 128:(g * RC + rc + 1) * 128],
                        ehat[:, rc:rc + 1],
                        start=False, stop=(rc == RC - 1),
                    )
            gates = sp.tile([128, 4], F32, tag="gates")
            for g, fn in ((0, AF.Sigmoid), (1, AF.Sigmoid), (2, AF.Tanh), (3, AF.Sigmoid)):
                nc.scalar.activation(
                    gates[:, g:g + 1], psA[:, g:g + 1], fn,
                    bias=xproj_sb[:, g * T + tm:g * T + tm + 1],
                )
            iu = sp.tile([128, 1], F32, tag="iu")
            nc.vector.tensor_mul(iu[:, :], gates[:, 0:1], gates[:, 2:3])
            ccol_new = sp.tile([128, 1], F32, tag="ccol")
            nc.vector.tensor_mul(ccol_new[:, :], gates[:, 3:4], ccol[:, :])
            nc.vector.tensor_add(ccol_new[:, :], ccol_new[:, :], iu[:, :])
            ccol = ccol_new
            tanh_c = sp.tile([128, 1], F32, tag="tanh_c")
            nc.scalar.activation(tanh_c[:, :], ccol[:, :], AF.Tanh)
            h_new = sp.tile([128, 1], F32, tag="h_new")
            hn_inst = nc.vector.tensor_mul(h_new[:, :], gates[:, 1:2], tanh_c[:, :])

            # w-gemv: w_partial = W1[own K]^T h_new
            psW = psp.tile([128, KC], F32, tag="psW")
            h_new_w = sp.tile([128, 1], wdt, tag="h_new_w")
            nc.vector.tensor_copy(h_new_w[:, :], h_new[:, :])
            for m in range(KC):
                nc.tensor.matmul(
                    psW[:, m:m + 1],
                    w1t_sb[:, m * 128:(m + 1) * 128],
                    h_new_w[:, :],
                    start=True, stop=True,
                )
            if t >= 2 and not no_comm:
                lw1 = nc.vector.wait_ge(lsem1, 128 * t)
                add_dep_helper(lw1.ins, hn_inst.ins, False, "anchor lsem1 wait")
            mm_inst = nc.vector.tensor_scalar_mul(pay1[par][:, 0:8], mask_sb[:, :], h_new[:, :])
            cp1 = nc.vector.tensor_copy(pay1[par][:, 8:16], psW[:, :])
            if t >= 2 and not no_comm:
                add_dep_helper(mm_inst.ins, lw1.ins, False, "pay1 WAR")
                add_dep_helper(cp1.ins, lw1.ins, False, "pay1 WAR")

            # ---- THE sync ----
            if not no_comm:
                _bcast_all(nc, lambda k: rec1[par][:, k * 16:(k + 1) * 16],
                           pay1[par][:, 0:16], sem1, lsem1, chain)
                w1_inst = nc.vector.wait_ge(sem1, 16 * (t + 1))
                add_dep_helper(w1_inst.ins, cp1.ins, False, "anchor sem1 wait")

            hnew_full = sp.tile([128, KC], F32, tag="hnew_full")
            r1 = rec1[par][:, :].rearrange("p (s c) -> p c s", s=N_CORES)
            i1 = nc.vector.reduce_sum(hnew_full[:, :], r1[:, 0:8, :], axis=mybir.AxisListType.X)
            w_sum = sp.tile([128, KC], F32, tag="w_sum")
            i2 = nc.vector.reduce_sum(w_sum[:, :], r1[:, 8:16, :], axis=mybir.AxisListType.X)
            if not no_comm:
                add_dep_helper(i1.ins, w1_inst.ins, False, "gate recv on sem1")
                add_dep_helper(i2.ins, w1_inst.ins, False, "gate recv on sem1")
            hnw = sp.tile([128, KC], wdt, tag="hnw")
            nc.vector.tensor_copy(hnw[:, :], hnew_full[:, :])

            # ---- replicated attention over all 512 rows ----
            tT = sp.tile([128, KC * MROWS], wdt, tag="tT")
            for c in range(KC):
                nc.scalar.activation(
                    tT[:, c * MROWS:(c + 1) * MROWS],
                    hw2T_sb[:, c * MROWS:(c + 1) * MROWS],
                    AF.Tanh, bias=w_sum[:, c:c + 1],
                )
            psS = psp.tile([128, RC], F32, tag="psS")
            for rc in range(RC):
                for c in range(KC):
                    nc.tensor.matmul(
                        psS[:, rc:rc + 1],
                        tT[:, c * MROWS + rc * 128:c * MROWS + (rc + 1) * 128],
                        wa_sb[:, c:c + 1],
                        start=(c == 0), stop=(c == KC - 1),
                    )
            # e = exp(score) = (1+tanh(s/2))/(1-tanh(s/2)); wa pre-scaled 0.5
            th = sp.tile([128, RC], F32, tag="th")
            nc.scalar.activation(th[:, :], psS[:, :], AF.Tanh)
            enum_t = sp.tile([128, RC], F32, tag="enum")
            nc.vector.tensor_scalar_add(enum_t[:, :], th[:, :], ones_sb[:, 0:1])
            eden = sp.tile([128, RC], F32, tag="eden")
            nc.vector.tensor_sub(eden[:, :], ones_sb[:, 0:RC], th[:, :])
            erec = sp.tile([128, RC], F32, tag="erec")
            nc.vector.reciprocal(erec[:, :], eden[:, :])
            e_t = sp.tile([128, RC], F32, tag="e_t")
            nc.vector.tensor_mul(e_t[:, :], enum_t[:, :], erec[:, :])
            # Z replicated across partitions
            zpart = sp.tile([128, 1], F32, tag="zpart")
            nc.vector.reduce_sum(zpart[:, :], e_t[:, :], axis=mybir.AxisListType.X)
            psZ = psp.tile([128, 1], F32, tag="psZ")
            nc.tensor.matmul(psZ[:, :], ones_sb[:, :], zpart[:, :], start=True, stop=True)
            rz = sp.tile([128, 1], F32, tag="rz")
            nc.vector.reciprocal(rz[:, :], psZ[:, :])
            ehf = sp.tile([128, RC], F32, tag="ehf")
            nc.vector.tensor_scalar_mul(ehf[:, :], e_t[:, :], rz[:, :])
            ehat = sp.tile([128, RC], wdt, tag="ehat")
            nc.vector.tensor_copy(ehat[:, :], ehf[:, :])

            # ---- own h_att slice = csum_own + h_new - (H[:, own]^T ê) ----
            psM = psp.tile([128, 1], F32, tag="psM")
            for rc in range(RC):
                nc.tensor.matmul(
                    psM[:, :],
                    h4own_sb[:, rc * 128:(rc + 1) * 128],
                    ehat[:, rc:rc + 1],
                    start=(rc == 0), stop=(rc == RC - 1),
                )
            hsl = sp.tile([128, 1], F32, tag="hsl")
            nc.vector.tensor_add(hsl[:, :], h_new[:, :], csown_sb[:, :])
            nc.vector.tensor_sub(hsl[:, :], hsl[:, :], psM[:, :])
            nc.sync.dma_start(hout.ap()[tm], hsl[:, :])

    nc.compile()
    return nc


def prep_in_maps(inputs: dict, T: int, wdt_np=None) -> list[dict]:
    """Host-side layout-only prep of per-core input maps."""
    if wdt_np is None:
        wdt_np = mybir.dt.np(BF16)
    X = np.asarray(inputs["inputs"], np.float32).reshape(T, IN_DIM)
    H = np.asarray(inputs["hiddn_state_mat"], np.float32)
    W_ioux = np.asarray(inputs["W_ioux"], np.float32)
    W_iouh = np.asarray(inputs["W_iouh"], np.float32)
    W_fx = np.asarray(inputs["W_fx"], np.float32)
    W_fh = np.asarray(inputs["W_fh"], np.float32)
    Wa = np.asarray(inputs["Wa"], np.float32).reshape(MEM)
    W_attnh = np.asarray(inputs["W_attnh"], np.float32)
    b_iou = (np.asarray(inputs["b_ioux"], np.float32)
             + np.asarray(inputs["b_iouh"], np.float32))
    b_f = (np.asarray(inputs["b_fx"], np.float32)
           + np.asarray(inputs["b_fh"], np.float32))
    b_attnh = np.asarray(inputs["b_attnh"], np.float32)

    W1 = W_attnh[:MEM]
    W2 = W_attnh[MEM:]

    xT_l = np.ascontiguousarray(
        X.T.reshape(KC, 128, T).transpose(1, 0, 2).reshape(128, KC * T))
    w2t = np.zeros((128, KC * KC * 128), np.float32)
    for c in range(KC):
        for k in range(KC):
            w2t[:, (c * KC + k) * 128:(c * KC + k + 1) * 128] = \
                W2[128 * k:128 * (k + 1), 128 * c:128 * (c + 1)]
    hT_l = np.ascontiguousarray(
        H.T.reshape(KC, 128, MROWS).transpose(1, 0, 2).reshape(128, KC * MROWS))
    # 0.5x so psS = score/2 (e' computed via tanh -- see kernel body)
    wa_l = (np.ascontiguousarray(Wa.reshape(KC, 128).T) * 0.5).astype(wdt_np)
    b2_l = np.ascontiguousarray(b_attnh.reshape(KC, 128).T)

    gate_w = [W_iouh[:, 0:MEM], W_iouh[:, MEM:2 * MEM], W_iouh[:, 2 * MEM:], W_fh]
    gate_wx = [W_ioux[:, 0:MEM], W_ioux[:, MEM:2 * MEM], W_ioux[:, 2 * MEM:], W_fx]
    gate_b = [b_iou[0:MEM], b_iou[MEM:2 * MEM], b_iou[2 * MEM:], b_f]

    maps = []
    for j in range(N_CORES):
        wrec = np.zeros((128, 4 * KC * 128), np.float32)
        wx = np.zeros((128, 4 * KC * 128), np.float32)
        for g in range(4):
            for k in range(KC):
                sl = np.s_[:, (g * KC + k) * 128:(g * KC + k + 1) * 128]
                wrec[sl] = gate_w[g][128 * k:128 * (k + 1), 128 * j:128 * (j + 1)]
                wx[sl] = gate_wx[g][128 * k:128 * (k + 1), 128 * j:128 * (j + 1)]
        w1t = np.zeros((128, KC * 128), np.float32)
        for m in range(KC):
            w1t[:, m * 128:(m + 1) * 128] = \
                W1[128 * j:128 * (j + 1), 128 * m:128 * (m + 1)]
        bias_x = np.stack([gate_b[g][128 * j:128 * (j + 1)] for g in range(4)], axis=1)
        mask = np.zeros((128, KC), np.float32)
        mask[:, j] = 1.0
        # h4own[p, rc*128+q] = H[128rc+p, 128j+q]
        h4own = np.ascontiguousarray(
            H[:, 128 * j:128 * (j + 1)].reshape(RC, 128, 128)
            .transpose(1, 0, 2).reshape(128, RC * 128))
        maps.append({
            "xT": xT_l, "wx": wx,
            "wrec": wrec.astype(wdt_np), "w1t": w1t.astype(wdt_np),
            "w2t": w2t, "hT": hT_l,
            "h4own": h4own.astype(wdt_np),
            "wa": wa_l, "bias_x": np.ascontiguousarray(bias_x),
            "bias2": b2_l, "mask": mask,
        })
    return maps


def assemble(houts: list[np.ndarray], T: int) -> np.ndarray:
    # houts[j] is [T, 128, 1]: h_att[t, 128j + p]
    out = np.empty((T, MEM), np.float32)
    for j in range(N_CORES):
        out[:, 128 * j:128 * (j + 1)] = np.asarray(houts[j]).reshape(T, 128)
    return out


# ----------------------------------------------------------------------------
def kernel(**inputs) -> np.ndarray:
    from concourse.bass_utils import run_bass_kernel_spmd

    T = int(np.asarray(inputs["inputs"]).shape[0])
    nc = build_nc(T)
    in_maps = prep_in_maps(inputs, T)
    res = run_bass_kernel_spmd(nc, in_maps, core_ids=list(range(N_CORES)))
    return assemble([res.results[j]["hout"] for j in range(N_CORES)], T)
